# revision 22
# baseline (speedup 1.0000x reference)
"""Masked multi-head attention block (B=4, N=1024, D=1024, H=16, DH=64) on 8
Trainium2 NeuronCores.

Sharding: core (b, g) = 2*b + g handles batch b and head-group g (8 of 16
heads). Each core computes qkv projections for its heads, attention, and its
partial output projection; the host sums the two head-group partials per batch.

Mask handling: the host gathers only the valid tokens per batch (padded to a
multiple of 128 with key-bias -30000 on the pad), so the device computes a
dense unmasked attention over ~half the sequence; invalid token rows of the
output are b_out.

v4 layout (per core, Vp = padded key count, Vq = padded query count):
  All inputs bf16.  x / wv / wo are packed on the host so each SBUF partition
  row is one contiguous DMA descriptor (10KB/8KB/8KB) instead of 8 small ones
  -- the input wall is descriptor-bound, not byte-bound.  wqk stays per-k so
  the pair-0 accumulation chain pipelines with chunk arrival.
  V'   [Vp, 8, 65]   values per head + ones column (softmax denominator)
  S^T  [Vp,Vq]/head  = K_chunk @ Q^T; exp on ACT with per-partition pad bias
  O^T  [65, Vq]/head = V'^T @ P^T accumulated over key chunks (row 64=denom)
  norm: one [65,Vq] PSUM->SBUF copy (numerators+denom, frees PSUM), K=1
        ones-matmul broadcasts the denom row across 64 partitions into PSUM,
        reciprocal_approx_fast on the wide tile (128-lane DVE), multiply.
  y    [Vq, D] = sum_h A_h @ w_out_h  (K=128 accumulating matmuls)

Scheduling: qk pair 0 and V' interleave k-outer through the DMA arrivals;
pairs 1-3 are background PE work drip-fed between attention chunks so the PE
never idles long enough for the HAM clock gate to re-throttle.
PSUM: prologue qkps 2 + vps 5 banks; stage 2 qkps 2 + stps 4 + otps 2.
"""
import json
import os
import sys

import numpy as np

sys.path.insert(0, "/opt/trn_rl_repo")

import concourse.bass as bass
import concourse.mybir as mybir
from concourse.tile import TileContext
from concourse import bass_utils

F32 = mybir.dt.float32
F32R = mybir.dt.float32r
BF16 = mybir.dt.bfloat16
AF = mybir.ActivationFunctionType

B, N, D, H, DH = 4, 1024, 1024, 16, 64
NCORES = 8
PAD_BIAS = -30000.0


def _install_patches():
    """The walrus build in this container accepts only one semaphore wait per
    instruction; hoist extra waits onto same-engine NoOps in the BIR json."""
    if getattr(bass.Bass, "_split_waits_patched", False):
        return
    orig = bass.Bass.to_json_bytes

    def to_json_bytes_split(self, *a, **k):
        j = json.loads(orig(self, *a, **k))
        for fn in j.get("functions", []):
            for bb in fn.get("blocks", []):
                out = []
                for ins in bb.get("instructions", []):
                    si = ins.get("sync_info") or {}
                    waits = si.get("on_wait") or []
                    if len(waits) > 1:
                        for i, w in enumerate(waits[:-1]):
                            out.append({
                                "debug": ins.get("debug", 0),
                                "engine": ins["engine"],
                                "ins": [],
                                "name": f"{ins['name']}_sw{i}",
                                "opcode": "NoOp",
                                "outs": [],
                                "text_hint": "splitw",
                                "sync_info": {"on_update": [], "on_wait": [w]},
                            })
                        si["on_wait"] = [waits[-1]]
                    out.append(ins)
                bb["instructions"] = out
        return json.dumps(j).encode()

    bass.Bass.to_json_bytes = to_json_bytes_split

    def _drain_and_barrier(self, tick_clock, wait_clock):
        import re as _re
        import bass_rust as _br
        from concourse.vector_clock import ScopedClock as _SC
        gc = tick_clock.global_clock
        comps = eval(_re.match(r"VectorClock\((\[.*\])\)", repr(gc)).group(1))
        for i, v in enumerate(comps):
            if v <= 0:
                continue
            sub = [0] * len(comps)
            sub[i] = v
            nop = self.nc.sync.nop(nofuse=True, hint="final_wait")
            wait_clock.add_sem_waits(nop.ins, _SC({None: _br.VectorClock(sub)}))
        self.nc.sync.drain()
        self.nc.all_engine_barrier()
        assert self.sems is not None
        popped = self.nc._tile_sem_poison_stack.pop()
        assert popped is self._sem_poison
        self.nc.clear_and_free_semaphores(list(self.sems.allocated().values()))

    TileContext._drain_and_barrier = _drain_and_barrier
    bass.Bass._split_waits_patched = True


def _build_program(Vp, Vq):
    KC = Vp // 128
    # key-side and query-side column slices: each slice gets its own PSUM
    # bank (matmul output must not cross a bank) and stays >= 256 wide.
    W = Vp if Vp <= 512 else Vp // 2
    QS = [(i * W, W) for i in range(Vp // W)]
    NQ = len(QS)
    Wq = Vq if Vq <= 512 else Vq // 2
    QSq = [(i * Wq, Wq) for i in range(Vq // Wq)]
    NQq = len(QSq)
    KCq = -(-Vq // 128)  # query-row chunks for the output projection
    NS = [(0, 512), (512, 512)]  # output D column halves

    nc = bass.Bass(trn_type="TRN2", target_bir_lowering=False, debug=False,
                   num_devices=NCORES)
    xt = nc.declare_dram_parameter("xt", [128, 8 * Vp], BF16,
                                   isOutput=False).ap()
    wqk = nc.declare_dram_parameter("wqk", [D, 1024], BF16, isOutput=False).ap()
    wv = nc.declare_dram_parameter("wv", [128, 8 * 512], BF16,
                                   isOutput=False).ap()
    wo = nc.declare_dram_parameter("wo", [128, 4 * 1024], BF16,
                                   isOutput=False).ap()
    biasv = nc.declare_dram_parameter("biasv", [128, KC], F32, isOutput=False).ap()
    onesr = nc.declare_dram_parameter("onesr", [1, 64], F32R, isOutput=False).ap()
    y = nc.declare_dram_parameter("y", [Vq, D], F32, isOutput=True).ap()

    with TileContext(nc) as tc:
        with tc.tile_pool(name="consts", bufs=1) as consts, \
             tc.tile_pool(name="xsb", bufs=1) as xpool, \
             tc.tile_pool(name="wqk", bufs=1) as wqkpool, \
             tc.tile_pool(name="wv", bufs=1) as wvpool, \
             tc.tile_pool(name="wo", bufs=1) as wopool, \
             tc.tile_pool(name="qk", bufs=1) as qkpool, \
             tc.tile_pool(name="vp", bufs=1) as vppool, \
             tc.tile_pool(name="pt", bufs=2 * KC + 10) as ptpool, \
             tc.tile_pool(name="at", bufs=1) as atpool, \
             tc.tile_pool(name="norm", bufs=4) as npool, \
             tc.tile_pool(name="ysb", bufs=3) as ypool:

            bias_sb = consts.tile([128, KC], F32)
            ones_sb = consts.tile([65, 64], F32R)
            nc.sync.dma_start(out=bias_sb[:], in_=biasv[:])
            # ones row lives at partition 64 so the K=1 broadcast matmul's
            # lhsT base partition matches the denominator row of osb.
            nc.sync.dma_start(out=ones_sb[64:65, :], in_=onesr[:])

            # ---- input DMAs.  scalar ring: x half 0, wv, x half 1, wo
            # (big merged descriptors); sync ring: wqk per-k (pipelines the
            # pair-0 accumulation chain). ----
            xsb_t = xpool.tile([128, 8, Vp], BF16, tag="x", name="x_all")
            wv_sb_t = wvpool.tile([128, 8, 512], BF16, tag="wv", name="wv_all")
            wo_sb_t = wopool.tile([128, 4, 1024], BF16, tag="wo", name="wo_all")
            nc.scalar.dma_start(out=xsb_t[:, 0:4, :], in_=xt[:, 0:4 * Vp])
            nc.scalar.dma_start(out=xsb_t[:, 4:8, :], in_=xt[:, 4 * Vp:8 * Vp])
            nc.scalar.dma_start(out=wv_sb_t[:], in_=wv[:])
            nc.scalar.dma_start(out=wo_sb_t[:], in_=wo[:])
            wqk_sb = []
            for k in range(8):
                wt = wqkpool.tile([128, 1024], BF16, tag=f"wqk{k}",
                                  name=f"wqk_{k}")
                nc.sync.dma_start(out=wt[:], in_=wqk[k * 128:(k + 1) * 128, :])
                wqk_sb.append(wt)

            at2 = [atpool.tile([128, Vq], BF16, tag=f"at{j}", name=f"at2_{j}")
                   for j in range(4)]
            qk_sb = [None] * 8
            vp_sb = [None] * KC

            # preload the ACT exp/ln table set during the DMA dead zone
            warm = npool.tile([1, 8], F32, tag="warm")
            nc.scalar.activation(out=warm[:], in_=bias_sb[0:1, 0:1]
                                 .broadcast_to([1, 8]), func=AF.Exp)

            # ---- prologue: qk pair 0, k-outer so the accumulation chains
            # pipeline with the per-k wqk DMA arrivals. ----
            with tc.tile_pool(name="p0ps", bufs=1, space="PSUM") as p0ps:
                p0t = {}
                for m in (0, 4):
                    mQS = QSq if m < 4 else QS
                    for qi, (n0, nw) in enumerate(mQS):
                        p0t[(m, qi)] = p0ps.tile([128, 512], F32,
                                                 tag=f"p0_{m}_{qi}",
                                                 name=f"p0_{m}_{qi}")
                for k in range(8):
                    for m in (0, 4):
                        mQS = QSq if m < 4 else QS
                        for qi, (n0, nw) in enumerate(mQS):
                            nc.tensor.matmul(
                                p0t[(m, qi)][:, 0:nw],
                                lhsT=wqk_sb[k][:, m * 128:(m + 1) * 128],
                                rhs=xsb_t[:, k, n0:n0 + nw],
                                start=(k == 0), stop=(k == 7))
                for m in (0, 4):
                    isq = m < 4
                    mV, mQS = (Vq, QSq) if isq else (Vp, QS)
                    qt = qkpool.tile([128, mV], BF16, tag=f"qk{m}",
                                     name=f"qk_{m}")
                    for qi, (n0, nw) in enumerate(mQS):
                        nc.vector.tensor_copy(out=qt[:, n0:n0 + nw],
                                              in_=p0t[(m, qi)][:, 0:nw])
                    qk_sb[m] = qt

            with tc.tile_pool(name="stps", bufs=2, space="PSUM") as stps, \
                 tc.tile_pool(name="otps", bufs=1, space="PSUM") as otps, \
                 tc.tile_pool(name="accps", bufs=2, space="PSUM") as accps:

                # background PE work items: V' chunks first (needed by the
                # first head's O^T), then qk pairs 1-3.  Each item is one
                # 8-matmul accumulation into a 1-bank ring tile + evac.
                bg = []

                def vp_item(c):
                    def run(c=c):
                        acc = accps.tile([128, 512], F32, tag="acc",
                                         name=f"vacc_{c}")
                        for k in range(8):
                            nc.tensor.matmul(
                                acc[:],
                                lhsT=xsb_t[:, k, c * 128:(c + 1) * 128],
                                rhs=wv_sb_t[:, k, :],
                                start=(k == 0), stop=(k == 7))
                        vt = vppool.tile([128, 8, 65], BF16, tag=f"vp{c}")
                        nc.vector.tensor_copy(
                            out=vt[:, :, 0:64],
                            in_=acc[:].rearrange("p (h d) -> p h d", h=8))
                        nc.gpsimd.memset(vt[:, :, 64:65], 1.0)
                        vp_sb[c] = vt
                    return run

                def pair_item(p, m, qi, n0, nw, mV):
                    def run():
                        if qk_sb[m] is None:
                            qk_sb[m] = qkpool.tile([128, mV], BF16,
                                                   tag=f"qk{m}",
                                                   name=f"qk_{m}")
                        acc = accps.tile([128, 512], F32, tag="acc",
                                         name=f"pacc_{m}_{qi}")
                        for k in range(8):
                            nc.tensor.matmul(
                                acc[:, 0:nw],
                                lhsT=wqk_sb[k][:, m * 128:(m + 1) * 128],
                                rhs=xsb_t[:, k, n0:n0 + nw],
                                start=(k == 0), stop=(k == 7))
                        nc.vector.tensor_copy(out=qk_sb[m][:, n0:n0 + nw],
                                              in_=acc[:, 0:nw])
                    return run

                def pair_items(p):
                    its = []
                    for m in (p, 4 + p):
                        mQS, mV = (QSq, Vq) if m < 4 else (QS, Vp)
                        for qi, (n0, nw) in enumerate(mQS):
                            its.append(pair_item(p, m, qi, n0, nw, mV))
                    return its
                for c in range(min(2, KC)):
                    bg.append(vp_item(c))
                bg.extend(pair_items(1))
                for c in range(2, KC):
                    bg.append(vp_item(c))
                bg.extend(pair_items(2))
                bg.extend(pair_items(3))
                bg_i = [0]

                def inject(n):
                    for _ in range(n):
                        if bg_i[0] < len(bg):
                            bg[bg_i[0]]()
                            bg_i[0] += 1

                # ---- stage 2: flat software pipeline over head pairs.
                # Head-units (S^T + exp) for (hp, 0) and (hp, 1) are emitted
                # inside hp-1's tail so ACT never drains at the boundary. ----
                HPS = [{"pts": {0: [None] * KC, 1: [None] * KC}, "ot": {},
                        "otc": 0} for _ in range(4)]

                def emit_ot(hp, sub, c):
                    S = HPS[hp]
                    for qi, (n0, nw) in enumerate(QSq):
                        nc.tensor.matmul(
                            S["ot"][sub][:, qi, 0:nw],
                            lhsT=vp_sb[c][:, 2 * hp + sub, :],
                            rhs=S["pts"][sub][c][:, n0:n0 + nw],
                            start=(c == 0), stop=(c == KC - 1))

                # for the last head pair, trail the DMA-needing odd half
                # inline and finish with the even half (direct at2 write) so
                # stage 3 is not blocked on the SBUF->SBUF DMA.
                FS = [0, 0, 0, 1]

                def head_unit(hp, c, trail=True):
                    S = HPS[hp]
                    qt = qk_sb[hp]
                    kt = qk_sb[4 + hp]
                    st = {}
                    for sub in (0, 1):
                        st[sub] = stps.tile([128, NQq, 512], F32, tag="st",
                                            name=f"st_{hp}_{sub}_{c}")
                    for qi, (n0, nw) in enumerate(QSq):
                        for sub in (0, 1):
                            lo = sub * 64
                            nc.tensor.matmul(
                                st[sub][:, qi, 0:nw],
                                lhsT=kt[lo:lo + 64, c * 128:(c + 1) * 128],
                                rhs=qt[lo:lo + 64, n0:n0 + nw],
                                start=True, stop=True,
                                tile_position=(lo, 0))
                    for sub in (0, 1):
                        pt = ptpool.tile([128, Vq], BF16, tag="pt",
                                         name=f"pt_{hp}_{sub}_{c}")
                        nc.scalar.activation(
                            out=pt[:].rearrange("p (q w) -> p q w", q=NQq),
                            in_=st[sub][:, :, 0:Wq], func=AF.Exp,
                            bias=bias_sb[:, c:c + 1], scale=1.0)
                        S["pts"][sub][c] = pt
                    if hp > 0 and trail and c >= 2:
                        fs = FS[hp]
                        if fs not in S["ot"]:
                            S["ot"][fs] = otps.tile([65, NQq, 512], F32,
                                                    tag="ot",
                                                    name=f"ot_{2 * hp + fs}")
                        while S["otc"] <= c - 2:
                            emit_ot(hp, fs, S["otc"])
                            S["otc"] += 1
                    inject(2)

                def denom_recip(hp, sub):
                    rln = npool.tile([65, Vq], F32, tag="rln")
                    nc.scalar.activation(
                        out=rln[64:65, :].rearrange("p (q w) -> p q w",
                                                    q=NQq),
                        in_=HPS[hp]["ot"][sub][64:65, :, 0:Wq], func=AF.Ln)
                    rex = npool.tile([65, Vq], F32R, tag="rex")
                    nc.scalar.activation(out=rex[64:65, :],
                                         in_=rln[64:65, :],
                                         func=AF.Exp, scale=-1.0)
                    return rex

                def osb_evac(hp, sub):
                    osb = npool.tile([64, Vq], F32, tag="osb")
                    nc.vector.tensor_copy(
                        out=osb[:].rearrange("p (q w) -> p q w", q=NQq),
                        in_=HPS[hp]["ot"][sub][0:64, :, 0:Wq])
                    return osb

                def bcast(hp, sub, rex):
                    rb = otps.tile([65, NQq, 512], F32, tag="ot",
                                   name=f"rb_{2 * hp + sub}")
                    for qi, (n0, nw) in enumerate(QSq):
                        nc.tensor.matmul(rb[0:64, qi, 0:nw],
                                         lhsT=ones_sb[64:65, :],
                                         rhs=rex[64:65, n0:n0 + nw],
                                         start=True, stop=True)
                    return rb

                def mul_at(hp, sub, osb, rb):
                    ov = osb[:].rearrange("p (q w) -> p q w", q=NQq)
                    rv = rb[0:64, :, 0:Wq]
                    if sub == 0:
                        nc.vector.tensor_mul(
                            at2[hp][0:64, :].rearrange("p (q w) -> p q w",
                                                       q=NQq), ov, rv)
                    else:
                        tmp = npool.tile([64, Vq], BF16, tag="odd")
                        nc.vector.tensor_mul(
                            tmp[:].rearrange("p (q w) -> p q w", q=NQq),
                            ov, rv)
                        nc.sync.dma_start(out=at2[hp][64:96, :],
                                          in_=tmp[0:32, :])
                        nc.sync.dma_start(out=at2[hp][96:128, :],
                                          in_=tmp[32:64, :])

                head_unit(0, 0)
                head_unit(0, 1)
                for hp in range(4):
                    S = HPS[hp]
                    fs, ss = FS[hp], 1 - FS[hp]
                    for c in range(2 if hp == 0 else min(3, KC), KC):
                        head_unit(hp, c)
                    # ---- tail, interleaved with hp+1's first head units ----
                    inject(3)
                    if hp == 0:
                        S["ot"][fs] = otps.tile([65, NQq, 512], F32,
                                                tag="ot", name="ot_0")
                        for c in range(KC):
                            emit_ot(hp, fs, c)
                    else:
                        for c in range(max(0, KC - 2), KC):
                            emit_ot(hp, fs, c)
                    rex0 = denom_recip(hp, fs)
                    osb0 = osb_evac(hp, fs)
                    rb0 = bcast(hp, fs, rex0)
                    mul_at(hp, fs, osb0, rb0)
                    S["ot"][ss] = otps.tile([65, NQq, 512], F32, tag="ot",
                                            name=f"ot_{2 * hp + ss}")
                    for c in range(min(2, KC)):
                        emit_ot(hp, ss, c)
                    if hp + 1 < 4:
                        head_unit(hp + 1, 0)
                    for c in range(2, KC):
                        emit_ot(hp, ss, c)
                        if c == 3 and hp + 1 < 4:
                            head_unit(hp + 1, 1)
                    rex1 = denom_recip(hp, ss)
                    osb1 = osb_evac(hp, ss)
                    if hp + 1 < 4 and KC > 2:
                        head_unit(hp + 1, 2, trail=False)
                    if hp == 3:
                        # weave the first y chunk's j=0..2 accumulations in
                        # while the last head's normalize chain drains.
                        mw0 = min(128, Vq)
                        ypre = {}
                        for (n0, nw) in NS:
                            yp = accps.tile([128, 512], F32, tag="acc",
                                            name=f"yp0_{n0}")
                            ypre[n0] = yp
                            for j in range(3):
                                nc.tensor.matmul(
                                    yp[0:mw0, 0:nw],
                                    lhsT=at2[j][:, 0:mw0],
                                    rhs=wo_sb_t[:, j, n0:n0 + nw],
                                    start=(j == 0), stop=False)
                    rb1 = bcast(hp, ss, rex1)
                    inject(1)
                    mul_at(hp, ss, osb1, rb1)
                    inject(1)
                inject(len(bg))

                # ---- stage 3: y[qc] = sum_j Apair_j @ wopair_j (K=128),
                # through the accps ring so it overlaps the last tail. ----
                mw0 = min(128, Vq)
                ysb0 = ypool.tile([128, 1024], F32, tag="ysb")
                for (n0, nw) in NS:
                    nc.tensor.matmul(ypre[n0][0:mw0, 0:nw],
                                     lhsT=at2[3][:, 0:mw0],
                                     rhs=wo_sb_t[:, 3, n0:n0 + nw],
                                     start=False, stop=True)
                    nc.vector.tensor_copy(out=ysb0[0:mw0, n0:n0 + nw],
                                          in_=ypre[n0][0:mw0, 0:nw])
                nc.scalar.dma_start(out=y[0:mw0, :], in_=ysb0[0:mw0, :])
                for qc in range(1, KCq):
                    mw = min(128, Vq - qc * 128)
                    ysb = ypool.tile([128, 1024], F32, tag="ysb")
                    for (n0, nw) in NS:
                        yp = accps.tile([128, 512], F32, tag="acc",
                                        name=f"yp_{qc}_{n0}")
                        for j in range(4):
                            nc.tensor.matmul(
                                yp[0:mw, 0:nw],
                                lhsT=at2[j][:, qc * 128:qc * 128 + mw],
                                rhs=wo_sb_t[:, j, n0:n0 + nw],
                                start=(j == 0), stop=(j == 3))
                        nc.vector.tensor_copy(out=ysb[0:mw, n0:n0 + nw],
                                              in_=yp[0:mw, 0:nw])
                    nc.scalar.dma_start(out=y[qc * 128:qc * 128 + mw, :],
                                        in_=ysb[0:mw, :])

    return nc


def kernel(x, mask, w_qkv, w_out, b_out):
    _install_patches()
    from concourse.bass_utils import run_bass_kernel_spmd

    x = np.asarray(x, dtype=np.float32)
    mask = np.asarray(mask, dtype=np.float32)
    w_qkv = np.asarray(w_qkv, dtype=np.float32)
    w_out = np.asarray(w_out, dtype=np.float32)
    b_out = np.asarray(b_out, dtype=np.float32)

    idx = [np.nonzero(mask[b] != 0.0)[0] for b in range(B)]
    nv = [len(i) for i in idx]
    Vp = max(128, int(-(-max(nv) // 128)) * 128)
    Vq = max(128, int(-(-max(nv) // 32)) * 32)
    if max(nv) == 0:
        return np.broadcast_to(b_out, (B, N, D)).astype(np.float32).copy()

    import ml_dtypes
    bf16 = ml_dtypes.bfloat16

    def pack_k(a, nk):
        # [nk*128, W] -> [128, nk*W]: partition p holds its nk chunks
        # contiguously, so the DMA moves one big descriptor per partition.
        W_ = a.shape[1]
        return np.ascontiguousarray(
            a.reshape(nk, 128, W_).transpose(1, 0, 2).reshape(128, nk * W_))

    scale = float(DH) ** -0.5
    G = 512  # features per head-group
    wqk_g, wv_g, wo_g = [], [], []
    for g in range(2):
        wq = w_qkv[:, g * G:(g + 1) * G] * scale
        wk = w_qkv[:, 1024 + g * G:1024 + (g + 1) * G]
        wqk_g.append(np.ascontiguousarray(
            np.concatenate([wq, wk], axis=1).astype(bf16)))
        wv_g.append(pack_k(
            w_qkv[:, 2048 + g * G:2048 + (g + 1) * G].astype(bf16), 8))
        wo_g.append(pack_k(w_out[g * G:(g + 1) * G, :].astype(bf16), 4))

    xt_b, bias_b = [], []
    for b in range(B):
        pad = Vp - nv[b]
        idxp = np.concatenate([idx[b], np.zeros(pad, dtype=np.int64)])
        xg = x[b][idxp, :]
        xt_b.append(pack_k(np.ascontiguousarray(xg.T.astype(bf16)), 8))
        bv = np.concatenate([
            np.zeros(nv[b], dtype=np.float32),
            np.full(pad, PAD_BIAS, dtype=np.float32)])
        bias_b.append(np.ascontiguousarray(bv.reshape(-1, 128).T))
    onesr = np.ones((1, 64), dtype=np.float32)

    nc = _build_program(Vp, Vq)
    in_maps = []
    for core in range(NCORES):
        b, g = core // 2, core % 2
        in_maps.append({
            "xt": xt_b[b], "wqk": wqk_g[g], "wv": wv_g[g], "wo": wo_g[g],
            "biasv": bias_b[b], "onesr": onesr,
        })

    trace = bool(os.environ.get("BASSK_TRACE"))
    if trace:
        _install_profile_hook()
    res = run_bass_kernel_spmd(nc, in_maps, list(range(NCORES)), trace=trace)
    global last_exec_time_ns
    last_exec_time_ns = res.exec_time_ns

    out = np.zeros((B, N, D), dtype=np.float32)
    for b in range(B):
        yb = res.results[2 * b]["y"] + res.results[2 * b + 1]["y"]
        out[b][idx[b]] = yb[:nv[b]]
    out += b_out
    return out


last_exec_time_ns = None


def _install_profile_hook():
    import types
    import antenv
    if 'antenv.axon_hooks' in sys.modules:
        return
    import trn_agent_boot.trn_boot as tb
    _hook = tb._ntff_profile_via_ctypes('/opt/axon/libaxon_pjrt.so')
    mod = types.ModuleType('antenv.axon_hooks')
    mod.get_axon_ntff_profile_hook = lambda: _hook
    mod.set_axon_ntff_profile_hook = lambda h: None
    sys.modules['antenv.axon_hooks'] = mod
    antenv.axon_hooks = mod
    bass_utils.upload_artifacts = lambda tmpdir: "local://skipped"
